# revision 74
# baseline (speedup 1.0000x reference)
"""AttentionPairBias Trainium2 Bass kernel — 8-core SPMD, block-sharded.

Sharding: 128 attention blocks -> 16 blocks (512 query rows) per core, with a
64-row halo on a/s so k/v windows need no cross-core exchange.

v3 layout of the hot paths:
- All bulk DMAs ride SP's hardware DGE (nc.sync); the 8 v-window builds ride
  Pool SWDGE so they don't head-of-line-block the z stream on SP.
- Pair-bias projection streams host-interleaved [z; z^2] fp8 pairs through
  DoubleRow matmuls with z stationary: each 128-position chunk lands in PSUM
  as [128 positions, 18] — already transposed. Evictions write an h-major
  [128, 18, NK] layout so downstream per-head reads are packed. Weights are
  scaled x64 into fp8 normal range; stats rescale on device. The z pipeline
  is interleaved into phases 1-2 via z_step() so DMA transfers hide under
  projection compute.
- q/k weights are host-padded to head-aligned [768, 1024] (2 heads per
  128-column tile at 64-offsets); 1/sqrt(dh) folded into wq on the host.
- Attention runs as two flat software-pipelined loops over 32 head-pairs:
  L1 (scores: qk + mean-sub + cb rank-1 + mask -> +bias -> exp) and L2
  (transpose, softmax sums via ones-matmul, AV, gated normalize), so no
  engine's in-order queue stalls behind a cross-engine latency chain.
- Softmax skips max-subtraction (scores are O(10)); normalization and the
  sigmoid gate are fused into one scalar_tensor_tensor per head.
"""
import math
import os
import sys
from contextlib import ExitStack

import numpy as np
import ml_dtypes

sys.path.insert(0, "/opt/trn_rl_repo")
sys.path.insert(0, "/opt/trn_rl_repo/concourse")

import concourse.bass as bass
import concourse.mybir as mybir
import concourse.tile as tile
from concourse import bacc, bass_utils
from concourse.masks import make_identity

B, N, CA, CS, CZ, H = 1, 4096, 768, 384, 128, 16
NQ, NK = 32, 128
DH = CA // H            # 48
NB = N // NQ            # 128
OFF = (NK - NQ) // 2    # 48
NCORE = 8
BPC = NB // NCORE       # 16 blocks per core
ROWS = BPC * NQ         # 512 own rows
HALO = 64
R = ROWS + 2 * HALO     # 640 rows incl. halo
NGRP = 4                # 4-block groups per core
RG = 4 * NQ * NK        # 16384 z-positions per group
EPS = 1e-5
ISCALE = 1.0 / math.sqrt(DH)
KA = CA // 128          # 6
KS = CS // 128          # 3
KH = H // 2             # 8 head-pair column tiles
CH = 4096               # z positions per streamed chunk
NCH = RG // CH          # 8 chunks per group
NZC = NGRP * NCH        # 32 chunks total
ZS = 64.0               # fp8 weight scale
NT = R // 128           # 5

FP32 = mybir.dt.float32
FP16 = mybir.dt.float16
FP8 = mybir.dt.float8e4
AF = mybir.ActivationFunctionType
ALU = mybir.AluOpType
DR = mybir.MatmulPerfMode.DoubleRow
NPF8 = ml_dtypes.float8_e4m3


def bcast_ap(dram, parts, n):
    """DRAM [n] -> AP [[0,parts],[1,n]] (partition broadcast)."""
    a = dram[:]
    return bass.AP(tensor=a.tensor, offset=a.offset, ap=[[0, parts], [1, n]])


def build_core_kernel(use_cb=True):
    nc = bacc.Bacc(None, target_bir_lowering=False)

    d_z8 = nc.dram_tensor("z8", [NGRP, CZ, 2, RG], FP8, kind="ExternalInput")
    d_a = nc.dram_tensor("a_h", [R, CA], FP16, kind="ExternalInput")
    d_s = nc.dram_tensor("s_h", [R, CS], FP16, kind="ExternalInput")
    d_wq = nc.dram_tensor("wq_pad", [CA, 128 * KH], FP8, kind="ExternalInput")
    d_wk = nc.dram_tensor("wk_pad", [CA, 128 * KH], FP8, kind="ExternalInput")
    d_wv = nc.dram_tensor("wv", [CA, CA], FP16, kind="ExternalInput")
    d_wg = nc.dram_tensor("wg", [CA, CA], FP16, kind="ExternalInput")
    d_wo = nc.dram_tensor("wo", [CA, CA], FP16, kind="ExternalInput")
    d_agw = nc.dram_tensor("adaln_g_w", [CS, CA], FP16, kind="ExternalInput")
    d_asw = nc.dram_tensor("adaln_s_w", [CS, CA], FP16, kind="ExternalInput")
    d_wl = nc.dram_tensor("w_last", [CS, CA], FP16, kind="ExternalInput")
    d_agb = nc.dram_tensor("adaln_g_b", [128, KA], FP32, kind="ExternalInput")
    d_bo = nc.dram_tensor("bo_b", [128, KA], FP32, kind="ExternalInput")
    d_bl = nc.dram_tensor("b_last_b", [128, KA], FP32, kind="ExternalInput")
    d_bgf = nc.dram_tensor("bg_full", [CA], FP16, kind="ExternalInput")
    d_wzdr = nc.dram_tensor("wzdr", [CZ, 2, 18], FP8, kind="ExternalInput")
    d_csI = nc.dram_tensor("csI", [128, H, 128], FP16, kind="ExternalInput")
    d_mask = nc.dram_tensor("mask_nq", [128, 2, NK], FP16, kind="ExternalInput")
    d_cbr = nc.dram_tensor("cb_row", [1, H, NK], FP16, kind="ExternalInput")
    d_out = nc.dram_tensor("outT", [CA, ROWS], FP16, kind="ExternalOutput")

    with tile.TileContext(nc) as tc, ExitStack() as ctx:
        const = ctx.enter_context(tc.tile_pool(name="const", bufs=1))
        pers = ctx.enter_context(tc.tile_pool(name="pers", bufs=1))
        ln_p = ctx.enter_context(tc.tile_pool(name="ln", bufs=2))
        at_p = ctx.enter_context(tc.tile_pool(name="attn", bufs=2))
        zs_p = ctx.enter_context(tc.tile_pool(name="zs", bufs=2))
        psA = ctx.enter_context(tc.tile_pool(name="psA", bufs=4, space="PSUM"))
        psB = ctx.enter_context(tc.tile_pool(name="psB", bufs=4, space="PSUM"))

        def pA(shape):
            return psA.tile(shape, FP32, tag="A", name="pA")

        def pB(shape, dt=FP32):
            return psB.tile(shape, dt, tag="B", name="pB")

        # ---------- constants ----------
        I16 = const.tile([128, 128], FP16, tag="I16")
        make_identity(nc, I16)
        wzdr = const.tile([CZ, 2, 18], FP8, tag="wzdr")
        nc.sync.dma_start(out=wzdr, in_=d_wzdr[:])
        csI = const.tile([128, H, 128], FP16, tag="csI")
        maskT = const.tile([128, 2, NK], FP16, tag="maskT")
        bob = const.tile([128, KA], FP32, tag="bob")
        blb = const.tile([128, KA], FP32, tag="blb")
        bg_bc = const.tile([128, CA], FP16, tag="bg_bc")
        cbrow = const.tile([1, H, NK], FP16, tag="cbrow")
        agb = const.tile([128, KA], FP32, tag="agb")
        nc.sync.dma_start(out=agb, in_=d_agb[:])
        epsb = const.tile([128, 1], FP32, tag="epsb")
        nc.vector.memset(epsb, EPS)
        ones1 = const.tile([1, 128], FP16, tag="ones1")
        nc.vector.memset(ones1, 1.0)
        onesc = const.tile([128, 1], FP16, tag="onesc")
        nc.vector.memset(onesc, 1.0)

        def load_w8(pool, dram, kt, name):
            t = pool.tile([128, kt, dram.shape[1]], FP8, tag=name)
            nc.sync.dma_start(out=t, in_=dram[:].rearrange("(k p) c -> p k c", p=128))
            return t

        def load_w(pool, dram, kt, name):
            t = pool.tile([128, kt, dram.shape[1]], FP16, tag=name)
            nc.sync.dma_start(out=t, in_=dram[:].rearrange("(k p) c -> p k c", p=128))
            return t

        # ---------- persistent activations ----------
        sT = pers.tile([128, KS, ROWS], FP16, tag="sT")
        qT_hd = pers.tile([128, KH, ROWS], FP16, tag="qT_hd")
        kT_hd = pers.tile([128, KH, R], FP16, tag="kT_hd")
        vw = pers.tile([128, BPC, CA], FP16, tag="vw")
        g_rm = pers.tile([128, NGRP, CA], FP16, tag="g_rm")
        go_rm = pers.tile([128, NGRP, CA], FP16, tag="go_rm")
        goT = pers.tile([128, KA, ROWS], FP16, tag="goT")
        gs16 = pers.tile([128, KA, ROWS], FP16, tag="gs16")
        bias_hm = pers.tile([128, NGRP, 18, NK], FP16, tag="bias_hm")
        p16_all = pers.tile([128, H * NGRP // 2, 256], FP16, tag="p16_all")
        mid_cm = tc.tile_pool(name="mid", bufs=1)
        mid = mid_cm.__enter__()
        a2T = mid.tile([128, KA, R], FP16, tag="a2T")
        a2T8 = mid.tile([128, KA, R], FP8, tag="a2T8")

        # ---------- interleaved z pipeline ----------
        # One step: DMA one [128, 2, CH] fp8 chunk (SP HWDGE), 16 DoubleRow
        # matmuls -> [128, 16, 18] PSUM, one transposing eviction into the
        # h-major bias tile. Steps are sprinkled through phases 1-2 so the z
        # HBM traffic hides under projection compute.
        zstate = {"i": 0, "ld": 0}
        zfifo = []

        def z_load():
            ld = zstate["ld"]
            if ld >= NZC:
                return
            zstate["ld"] = ld + 1
            g, cc = ld // NCH, ld % NCH
            zc = zs_p.tile([128, 2, CH], FP8, tag="zc")
            nc.sync.dma_start(out=zc, in_=d_z8[g, :, :, cc * CH : (cc + 1) * CH])
            zfifo.append(zc)

        def z_step(n=1):
            for _ in range(n):
                i = zstate["i"]
                if i >= NZC:
                    return
                zstate["i"] = i + 1
                g, cc = i // NCH, i % NCH
                if not zfifo:
                    z_load()
                zc = zfifo.pop(0)
                z_load()
                nsub = CH // 128
                half = nsub // 2
                for hi in range(2):
                    pz = pB([128, half, 18])
                    for k in range(half):
                        kk = hi * half + k
                        nc.tensor.matmul(
                            pz[:, k, :], zc[:, :, kk * 128 : (kk + 1) * 128], wzdr,
                            start=True, stop=True, perf_mode=DR,
                        )
                    k0 = cc * nsub + hi * half
                    if (2 * i + hi) % 4 < 1:
                        nc.vector.tensor_copy(
                            out=bias_hm[:, g, :, k0 : k0 + half],
                            in_=pz.rearrange("p k c -> p c k"),
                        )
                    else:
                        nc.scalar.activation(
                            out=bias_hm[:, g, :, k0 : k0 + half],
                            in_=pz.rearrange("p k c -> p c k"), func=AF.Copy,
                        )

        # ================= Phase 1: LN + adaln -> a2T =================
        with tc.tile_pool(name="ph1", bufs=1) as ph1:
            w_ag = load_w(ph1, d_agw, KS, "w_ag")
            w_as = load_w(ph1, d_asw, KS, "w_as")
            s_lnT = ph1.tile([128, KS, R], FP16, tag="s_lnT")
            a_lnT = ph1.tile([128, KA, R], FP16, tag="a_lnT")

            # Pass 1: load all row tiles, accumulate bn stats; ONE Sqrt for
            # every tile (keeps the Act table in one function era), then
            # normalize + transpose per tile in pass 2.
            srcs = [(d_s, CS, KS, s_lnT), (d_a, CA, KA, a_lnT)]
            xs = {}
            mv_all = ph1.tile([128, 2, NT, 2], FP32, tag="mv_all")
            for t in range(NT):
                for wi, (dram, width, kt, dst) in enumerate(srcs):
                    x = ph1.tile([128, width], FP16, tag=f"ln_x{width}", bufs=NT)
                    nc.sync.dma_start(out=x, in_=dram[t * 128 : (t + 1) * 128, :])
                    nsub = width // 384
                    st = ph1.tile([128, nsub, 6], FP32, tag=f"ln_st{width}", bufs=2)
                    for j in range(nsub):
                        nc.vector.bn_stats(
                            out=st[:, j, :], in_=x[:, j * 384 : (j + 1) * 384]
                        )
                    nc.vector.bn_aggr(out=mv_all[:, wi, t, :], in_=st)
                    xs[(wi, t)] = x
                z_step()
            rstd_ln = ph1.tile([128, 2, NT], FP32, tag="rstd_ln")
            nc.scalar.activation(
                out=rstd_ln, in_=mv_all[:, :, :, 1], func=AF.Sqrt, bias=epsb
            )
            nc.vector.reciprocal(out=rstd_ln, in_=rstd_ln)
            for t in range(NT):
                for wi, (dram, width, kt, dst) in enumerate(srcs):
                    x16 = xs[(wi, t)]
                    nc.vector.tensor_scalar(
                        out=x16, in0=x16, scalar1=mv_all[:, wi, t, 0:1],
                        scalar2=rstd_ln[:, wi, t : t + 1],
                        op0=ALU.subtract, op1=ALU.mult,
                    )
                    pt = pB([128, kt, 128], FP16)
                    for k in range(kt):
                        nc.tensor.transpose(
                            pt[:, k, :], x16[:, k * 128 : (k + 1) * 128], I16
                        )
                    nc.vector.tensor_copy(
                        out=dst[:, :, t * 128 : (t + 1) * 128], in_=pt
                    )
                z_step()

            # raw s transposed (own rows) for the final gate
            for t in range(ROWS // 128):
                x = ph1.tile([128, CS], FP16, tag="s_raw", bufs=2)
                nc.sync.dma_start(
                    out=x, in_=d_s[HALO + t * 128 : HALO + (t + 1) * 128, :]
                )
                pt = pB([128, KS, 128], FP16)
                for k in range(KS):
                    nc.tensor.transpose(pt[:, k, :], x[:, k * 128 : (k + 1) * 128], I16)
                nc.vector.tensor_copy(out=sT[:, :, t * 128 : (t + 1) * 128], in_=pt)

            RC = 320
            for co in range(KA):
                for rc in range(R // RC):
                    rs = slice(rc * RC, (rc + 1) * RC)
                    pg = pA([128, RC])
                    pv = pA([128, RC])
                    for k in range(KS):
                        nc.tensor.matmul(
                            pg, (w_ag[:, k, co * 128 : (co + 1) * 128]),
                            (s_lnT[:, k, rs]), start=(k == 0), stop=(k == KS - 1),
                        )
                    for k in range(KS):
                        nc.tensor.matmul(
                            pv, (w_as[:, k, co * 128 : (co + 1) * 128]),
                            (s_lnT[:, k, rs]), start=(k == 0), stop=(k == KS - 1),
                        )
                    gate = ph1.tile([128, RC], FP32, tag="gate", bufs=2)
                    nc.scalar.activation(
                        out=gate, in_=pg, func=AF.Sigmoid, bias=agb[:, co : co + 1]
                    )
                    nc.vector.tensor_mul(gate, gate, a_lnT[:, co, rs])
                    nc.vector.tensor_add(a2T[:, co, rs], gate, pv)
                    nc.gpsimd.tensor_copy(out=a2T8[:, co, rs], in_=a2T[:, co, rs])
                    z_step()

        nc.sync.dma_start(out=blb, in_=d_bl[:])
        # ============ Phase 1b: final-gate projection (sigmoid era) ==========
        with tc.tile_pool(name="ph1b", bufs=1) as ph1b:
            w_l = load_w(ph1b, d_wl, KS, "w_l")
            for co in range(KA):
                pl = pA([128, ROWS])
                for k in range(KS):
                    nc.tensor.matmul(
                        pl, (w_l[:, k, co * 128 : (co + 1) * 128]),
                        (sT[:, k, :]), start=(k == 0), stop=(k == KS - 1),
                    )
                nc.scalar.activation(
                    out=gs16[:, co, :], in_=pl, func=AF.Sigmoid,
                    bias=blb[:, co : co + 1],
                )
                z_step()

        # ================= Phase 2a: q/k projections (head-aligned) ==========
        with tc.tile_pool(name="ph2a", bufs=1) as ph2a:
            w_q = load_w8(ph2a, d_wq, KA, "w_q")
            w_k = load_w8(ph2a, d_wk, KA, "w_k")
            wq4 = w_q.rearrange("p (kp two) c -> p kp two c", two=2)
            wk4 = w_k.rearrange("p (kp two) c -> p kp two c", two=2)
            a84 = a2T8.rearrange("p (kp two) r -> p kp two r", two=2)
            for co in range(KH):
                pq = [pA([128, 256]) for _ in range(2)]
                pk = [pA([128, 320]) for _ in range(2)]
                for kp in range(KA // 2):
                    for rc in range(2):
                        rs = slice(HALO + rc * 256, HALO + (rc + 1) * 256)
                        nc.tensor.matmul(
                            pq[rc], (wq4[:, kp, :, co * 128 : (co + 1) * 128]),
                            (a84[:, kp, :, rs]), start=(kp == 0),
                            stop=(kp == KA // 2 - 1), perf_mode=DR,
                        )
                for rc in range(2):
                    nc.scalar.activation(
                        out=qT_hd[:, co, rc * 256 : (rc + 1) * 256], in_=pq[rc],
                        func=AF.Copy, scale=1.0 / ZS,
                    )
                for kp in range(KA // 2):
                    for rc in range(2):
                        rs = slice(rc * 320, (rc + 1) * 320)
                        nc.tensor.matmul(
                            pk[rc], (wk4[:, kp, :, co * 128 : (co + 1) * 128]),
                            (a84[:, kp, :, rs]), start=(kp == 0),
                            stop=(kp == KA // 2 - 1), perf_mode=DR,
                        )
                if co % 2 == 0:
                    for rc in range(2):
                        nc.scalar.activation(
                            out=kT_hd[:, co, rc * 320 : (rc + 1) * 320], in_=pk[rc],
                            func=AF.Copy, scale=1.0 / ZS,
                        )
                else:
                    for rc in range(2):
                        nc.vector.tensor_scalar(
                            out=kT_hd[:, co, rc * 320 : (rc + 1) * 320], in0=pk[rc],
                            scalar1=1.0 / ZS, scalar2=None, op0=ALU.mult,
                        )
                z_step()

        nc.sync.dma_start(out=bg_bc, in_=bcast_ap(d_bgf, 128, CA))
        # ================= Phase 2b: v (row-major) + g =================
        with tc.tile_pool(name="ph2b", bufs=1) as ph2b:
            w_v = load_w(ph2b, d_wv, KA, "w_v")
            w_g = load_w(ph2b, d_wg, KA, "w_g")
            v_rm = ph2b.tile([128, NT, CA], FP16, tag="v_rm")
            for rt in range(NT):
                for c2 in range(2):
                    pv = pA([128, 384])
                    for k in range(KA):
                        nc.tensor.matmul(
                            pv, (a2T[:, k, rt * 128 : (rt + 1) * 128]),
                            (w_v[:, k, c2 * 384 : (c2 + 1) * 384]),
                            start=(k == 0), stop=(k == KA - 1),
                        )
                    nc.vector.tensor_copy(
                        out=v_rm[:, rt, c2 * 384 : (c2 + 1) * 384], in_=pv
                    )
                z_step(2)
            for rt in range(ROWS // 128):
                for c2 in range(2):
                    pg = pA([128, 384])
                    for k in range(KA):
                        nc.tensor.matmul(
                            pg, (a2T[:, k, HALO + rt * 128 : HALO + (rt + 1) * 128]),
                            (w_g[:, k, c2 * 384 : (c2 + 1) * 384]),
                            start=(k == 0), stop=(k == KA - 1),
                        )
                    nc.vector.tensor_add(pg, pg, bg_bc[:, c2 * 384 : (c2 + 1) * 384])
                    nc.scalar.activation(
                        out=g_rm[:, rt, c2 * 384 : (c2 + 1) * 384], in_=pg,
                        func=AF.Sigmoid,
                    )
                z_step()
            # per-block key/value windows of v: 4 batched strided copies per
            # piece (blocks n=4t+j share partition offset p0=16+32j, t=0..3).
            # Pool SWDGE so these don't block the z stream on SP's queue.
            vw4 = vw.rearrange("p (t j) c -> p t j c", j=4)
            for j in range(4):
                p0 = 16 + 32 * j
                n0 = 128 - p0
                nc.gpsimd.dma_start(out=vw4[0:n0, :, j, :], in_=v_rm[p0:128, 0:4, :])
                nc.gpsimd.dma_start(out=vw4[n0:128, :, j, :], in_=v_rm[0:p0, 1:5, :])
            z_step(NZC)  # drain whatever z remains

        mid_cm.__exit__(None, None, None)

        nc.sync.dma_start(out=csI, in_=d_csI[:])
        nc.sync.dma_start(out=maskT, in_=d_mask[:])
        nc.sync.dma_start(out=cbrow, in_=d_cbr[:])
        nc.sync.dma_start(out=bob, in_=d_bo[:])
        w_o = load_w(pers, d_wo, KA, "w_o")
        fin_sb = pers.tile([128, KA, ROWS], FP16, tag="fin_sb")

        # ================= Phase 3: group stats =================
        # mu/ex2 from the x64-scaled sum columns; rstd via Rsqrt (one act
        # table load for all four groups); rep2 is rstd/64 duplicated per
        # head-pair lane for the packed th multiply.
        murs, reps = [], []
        mus = []
        var_all = at_p.tile([128, NGRP, NK], FP32, tag="var_all", bufs=1)
        for g in range(NGRP):
            mu = at_p.tile([128, NK], FP32, tag="mu", bufs=4)
            nc.scalar.mul(out=mu, in_=bias_hm[:, g, 16, :], mul=1.0 / (ZS * CZ))
            ex2 = at_p.tile([128, NK], FP32, tag="ex2")
            nc.scalar.mul(out=ex2, in_=bias_hm[:, g, 17, :], mul=1.0 / (ZS * CZ))
            nc.vector.tensor_mul(var_all[:, g, :], mu, mu)
            nc.vector.tensor_sub(var_all[:, g, :], ex2, var_all[:, g, :])
            mus.append(mu)
        rstd_all = at_p.tile([128, NGRP, NK], FP32, tag="rstd_all", bufs=1)
        nc.scalar.activation(out=rstd_all, in_=var_all, func=AF.Sqrt, bias=epsb)
        nc.vector.reciprocal(out=rstd_all, in_=rstd_all)
        for g in range(NGRP):
            mur = at_p.tile([128, NK], FP16, tag="mur", bufs=4)
            nc.vector.tensor_mul(mur, mus[g], rstd_all[:, g, :])
            rep2 = at_p.tile([128, 2, NK], FP16, tag="rep2", bufs=4)
            nc.scalar.mul(out=rep2[:, 0, :], in_=rstd_all[:, g, :], mul=1.0 / ZS)
            nc.scalar.mul(out=rep2[:, 1, :], in_=rstd_all[:, g, :], mul=1.0 / ZS)
            murs.append(mur)
            reps.append(rep2)

        # ================= Phase 4: attention L1 — scores -> p16 ============
        pairs = [(g, hp) for g in range(NGRP) for hp in range(KH)]
        for pi, (g, hp) in enumerate(pairs):
            edge = g in (0, NGRP - 1)
            S2 = pB([128, 2, NK])
            for j in range(2):
                h = 2 * hp + j
                o64 = (h % 2) * 64
                for nn in range(4):
                    n = g * 4 + nn
                    wlo = 16 + 32 * n
                    nc.tensor.matmul(
                        S2[nn * NQ : (nn + 1) * NQ, j, :],
                        qT_hd[o64 : o64 + DH, hp, n * NQ : (n + 1) * NQ],
                        kT_hd[o64 : o64 + DH, hp, wlo : wlo + NK],
                        start=True, stop=False,
                        tile_position=(o64, nn * NQ),
                    )
                nc.tensor.matmul(
                    S2[:, j, :], csI[:, h, :], murs[g], start=False,
                    stop=not (use_cb or edge),
                )
                if use_cb:
                    nc.tensor.matmul(
                        S2[:, j, :], ones1, cbrow[:, h, :], start=False,
                        stop=not edge,
                    )
                if edge:
                    nc.tensor.matmul(
                        S2[:, j, :], I16, maskT[:, 0 if g == 0 else 1, :],
                        start=False, stop=True,
                    )
            th2 = at_p.tile([128, 2, NK], FP16, tag="th2", bufs=3)
            nc.vector.tensor_mul(th2, bias_hm[:, g, 2 * hp : 2 * hp + 2, :], reps[g])
            nc.vector.tensor_add(S2, S2, th2)
            nc.scalar.activation(out=p16_all[:, pi, :], in_=S2, func=AF.Exp)

        # ============ Phase 5: attention L2 — transpose, sums, AV ============
        # One-stage software pipeline: pair p's transpose+evict issues before
        # pair p-1's sums/AV, so PE never stalls behind the Act pT2 copy.
        l2state = {}
        l2fin = []
        pT2s = {}

        def l2_front(pi):
            g, hp = pairs[pi]
            ptp2 = pB([128, 2, 128], FP16)
            p16v = p16_all[:, pi, :].rearrange("p (j k) -> p j k", j=2)
            for j in range(2):
                nc.tensor.transpose(ptp2[:, j, :], p16v[:, j, :], I16)
            pT2 = at_p.tile([128, 2, 128], FP16, tag="pT2", bufs=4)
            nc.scalar.activation(out=pT2, in_=ptp2, func=AF.Copy)
            pT2s[pi] = pT2

        def l2_back(pi):
            g, hp = pairs[pi]
            pT2 = pT2s.pop(pi)
            if pi % 2 == 0:
                sums4 = pA([128, 4])
                rec4 = at_p.tile([128, 4], FP32, tag="rec4", bufs=3)
                l2state["sums4"], l2state["rec4"] = sums4, rec4
            else:
                sums4, rec4 = l2state["sums4"], l2state["rec4"]
            sbase = (pi % 2) * 2
            ov2 = pA([128, 2, DH])
            for j in range(2):
                h = 2 * hp + j
                nc.tensor.matmul(
                    sums4[:, sbase + j : sbase + j + 1], pT2[:, j, :], onesc,
                    start=True, stop=True,
                )
                for nn in range(4):
                    n = g * 4 + nn
                    nc.tensor.matmul(
                        ov2[nn * NQ : (nn + 1) * NQ, j, :],
                        pT2[:, j, nn * NQ : (nn + 1) * NQ],
                        vw[:, n, h * DH : (h + 1) * DH],
                        start=True, stop=True,
                        tile_position=(0, nn * NQ),
                    )
            if pi % 2 == 1:
                nc.vector.reciprocal(out=rec4, in_=sums4)
            l2fin.append((pi, g, hp, ov2, rec4, sbase))
            while l2fin and (l2fin[0][0] < pi or pi == len(pairs) - 1):
                fpi, fg, fhp, fov2, frec4, fsb = l2fin.pop(0)
                for j in range(2):
                    h = 2 * fhp + j
                    nc.vector.scalar_tensor_tensor(
                        out=go_rm[:, fg, h * DH : (h + 1) * DH], in0=fov2[:, j, :],
                        scalar=frec4[:, fsb + j : fsb + j + 1],
                        in1=g_rm[:, fg, h * DH : (h + 1) * DH],
                        op0=ALU.mult, op1=ALU.mult,
                    )

        for pi in range(len(pairs)):
            l2_front(pi)
            if pi >= 2:
                l2_back(pi - 2)
        l2_back(len(pairs) - 2)
        l2_back(len(pairs) - 1)

        # ================= Phase 6: (g*o) transpose =================
        for g in range(NGRP):
            pt = pB([128, KA, 128], FP16)
            for k in range(KA):
                nc.tensor.transpose(pt[:, k, :], go_rm[:, g, k * 128 : (k + 1) * 128], I16)
            nc.vector.tensor_copy(out=goT[:, :, g * 128 : (g + 1) * 128], in_=pt)

        # ================= Phase 7: output projection =================
        for co in range(KA):
            po = pA([128, ROWS])
            for k in range(KA):
                nc.tensor.matmul(
                    po, (w_o[:, k, co * 128 : (co + 1) * 128]),
                    (goT[:, k, :]), start=(k == 0), stop=(k == KA - 1),
                )
            nc.vector.scalar_tensor_tensor(
                out=fin_sb[:, co, :], in0=po, scalar=bob[:, co : co + 1],
                in1=gs16[:, co, :], op0=ALU.add, op1=ALU.mult,
            )

        # ================= Phase 7 tail: stores =================
        for co in range(KA):
            nc.sync.dma_start(
                out=d_out[co * 128 : (co + 1) * 128, :], in_=fin_sb[:, co, :]
            )

    nc.compile()
    return nc


def host_prep(inputs):
    a = np.ascontiguousarray(np.asarray(inputs["a"], np.float32)[0])
    s = np.ascontiguousarray(np.asarray(inputs["s"], np.float32)[0])
    z = np.asarray(inputs["z"], np.float32)[0]
    gz = np.asarray(inputs["gz"], np.float32)
    bz = np.asarray(inputs["bz"], np.float32)
    wz = np.asarray(inputs["wz"], np.float32)
    wz2 = gz[:, None] * wz
    cs = wz2.sum(0)
    cb = (bz @ wz).astype(np.float32)

    # DoubleRow pair-bias weights: [CZ, 2, 18] fp8, x64 for fp8 range.
    # Slot 0 pairs with z (head cols + sum col), slot 1 with z^2 (sum col).
    wzdr = np.zeros((CZ, 2, 18), np.float32)
    wzdr[:, 0, :H] = wz2 * ZS
    wzdr[:, 0, 16] = ZS
    wzdr[:, 1, 17] = ZS
    wzdr = wzdr.astype(NPF8)

    csI = np.zeros((128, H, 128), np.float16)
    for h in range(H):
        for p in range(128):
            csI[p, h, p] = np.float16(-cs[h])

    cb_row = np.zeros((1, H, NK), np.float16)
    cb_row[0, :, :] = cb[:, None]

    # head-aligned padded projection weights: head h -> cols (h//2)*128 +
    # (h%2)*64 + [0,48); 1/sqrt(dh) folded into wq
    def pad_heads(w, scale=1.0):
        wp = np.zeros((CA, 128 * KH), np.float32)
        for h in range(H):
            dst = (h // 2) * 128 + (h % 2) * 64
            wp[:, dst : dst + DH] = w[:, h * DH : (h + 1) * DH] * scale
        return wp.astype(np.float16)

    def btile(v):
        return np.ascontiguousarray(v.reshape(KA, 128).T.astype(np.float32))

    common = {
        "wq_pad": pad_heads(np.asarray(inputs["wq"], np.float32), ISCALE * ZS).astype(np.float32).astype(NPF8),
        "wk_pad": pad_heads(np.asarray(inputs["wk"], np.float32), ZS).astype(np.float32).astype(NPF8),
        "wv": np.asarray(inputs["wv"], np.float16),
        "wg": np.asarray(inputs["wg"], np.float16),
        "wo": np.asarray(inputs["wo"], np.float16),
        "adaln_g_w": np.asarray(inputs["adaln_g_w"], np.float16),
        "adaln_s_w": np.asarray(inputs["adaln_s_w"], np.float16),
        "w_last": np.asarray(inputs["w_last"], np.float16),
        "adaln_g_b": btile(np.asarray(inputs["adaln_g_b"], np.float32)),
        "bo_b": btile(np.asarray(inputs["bo"], np.float32)),
        "b_last_b": btile(np.asarray(inputs["b_last"], np.float32)),
        "bg_full": np.asarray(inputs["bg"], np.float16),
        "wzdr": wzdr, "csI": csI, "cb_row": cb_row,
    }

    in_maps = []
    for c in range(NCORE):
        lo = c * ROWS - HALO
        hi = c * ROWS + ROWS + HALO
        a_h = np.zeros((R, CA), np.float16)
        s_h = np.zeros((R, CS), np.float16)
        g0, g1 = max(lo, 0), min(hi, N)
        a_h[g0 - lo : g1 - lo] = a[g0:g1]
        s_h[g0 - lo : g1 - lo] = s[g0:g1]
        z_c = z[c * BPC : (c + 1) * BPC]
        zg = z_c.reshape(NGRP, 4, NQ, NK, CZ)
        # [NGRP, CZ, (k, n, q)] position-major, fp8; z^2 from the fp8-rounded z
        zT = zg.transpose(0, 4, 3, 1, 2).reshape(NGRP, CZ, RG).astype(NPF8)
        z8 = np.empty((NGRP, CZ, 2, RG), NPF8)
        z8[:, :, 0, :] = zT
        z8[:, :, 1, :] = (zT.astype(np.float32) ** 2).astype(NPF8)
        z8 = np.ascontiguousarray(z8)
        nglob = c * BPC + np.arange(BPC)
        idx = nglob[:, None] * NQ + np.arange(NK)[None, :] - OFF
        mask = np.where((idx >= 0) & (idx < N), 0.0, -30000.0).astype(np.float32)
        # edge groups only: [128(4n,32q), {first,last}, NK]
        mask_g = (
            np.repeat(mask.reshape(NGRP, 4, 1, NK), NQ, axis=2)
            .reshape(NGRP, 128, NK)
            .transpose(1, 0, 2)
            .astype(np.float16)
        )
        mask_nq = np.ascontiguousarray(mask_g[:, [0, NGRP - 1], :])
        m = dict(common)
        m.update({"z8": z8, "a_h": a_h, "s_h": s_h, "mask_nq": mask_nq})
        in_maps.append(m)
    return in_maps


_NC_CACHE = {}


def kernel(**inputs):
    use_cb = bool(
        np.any(np.asarray(inputs["bz"], np.float32) @ np.asarray(inputs["wz"], np.float32))
    )
    key = ("nc", use_cb)
    if key not in _NC_CACHE:
        _NC_CACHE[key] = build_core_kernel(use_cb=use_cb)
    nc = _NC_CACHE[key]
    in_maps = host_prep(inputs)
    res = bass_utils.run_bass_kernel_spmd(
        nc, in_maps, core_ids=list(range(NCORE)),
        trace=bool(int(os.environ.get("KTRACE", "0"))),
    )
    kernel.last_results = res
    outs = [np.asarray(res.results[c]["outT"]).T for c in range(NCORE)]
    return np.ascontiguousarray(np.concatenate(outs, 0)[None]).astype(np.float32)


# revision 102
# speedup vs baseline: 1.1468x; 1.1468x over previous
"""AttentionPairBias Trainium2 Bass kernel — 8-core SPMD, block-sharded.

Sharding: 128 attention blocks -> 16 blocks (512 query rows) per core, with a
64-row halo on a/s so k/v windows need no cross-core exchange.

v3 layout of the hot paths:
- All bulk DMAs ride SP's hardware DGE (nc.sync); the 8 v-window builds ride
  Pool SWDGE so they don't head-of-line-block the z stream on SP.
- Pair-bias projection streams host-interleaved [z; z^2] fp8 pairs through
  DoubleRow matmuls with z stationary: each 128-position chunk lands in PSUM
  as [128 positions, 18] — already transposed. Evictions write an h-major
  [128, 18, NK] layout so downstream per-head reads are packed. Weights are
  scaled x64 into fp8 normal range; stats rescale on device. The z pipeline
  is interleaved into phases 1-2 via z_step() so DMA transfers hide under
  projection compute.
- q/k weights are host-padded to head-aligned [768, 1024] (2 heads per
  128-column tile at 64-offsets); 1/sqrt(dh) folded into wq on the host.
- Attention runs as two flat software-pipelined loops over 32 head-pairs:
  L1 (scores: qk + mean-sub + cb rank-1 + mask -> +bias -> exp) and L2
  (transpose, softmax sums via ones-matmul, AV, gated normalize), so no
  engine's in-order queue stalls behind a cross-engine latency chain.
- Softmax skips max-subtraction (scores are O(10)); normalization and the
  sigmoid gate are fused into one scalar_tensor_tensor per head.
"""
import math
import os
import sys
from contextlib import ExitStack

import numpy as np
import ml_dtypes

sys.path.insert(0, "/opt/trn_rl_repo")
sys.path.insert(0, "/opt/trn_rl_repo/concourse")

import concourse.bass as bass
import concourse.mybir as mybir
import concourse.tile as tile
from concourse import bacc, bass_utils
from concourse.masks import make_identity

B, N, CA, CS, CZ, H = 1, 4096, 768, 384, 128, 16
NQ, NK = 32, 128
DH = CA // H            # 48
NB = N // NQ            # 128
OFF = (NK - NQ) // 2    # 48
NCORE = 8
BPC = NB // NCORE       # 16 blocks per core
ROWS = BPC * NQ         # 512 own rows
HALO = 64
R = ROWS + 2 * HALO     # 640 rows incl. halo
NGRP = 4                # 4-block groups per core
RG = 4 * NQ * NK        # 16384 z-positions per group
EPS = 1e-5
ISCALE = 1.0 / math.sqrt(DH)
KA = CA // 128          # 6
KS = CS // 128          # 3
KH = H // 2             # 8 head-pair column tiles
CH = 2048               # z positions per streamed chunk
NCH = RG // CH          # 8 chunks per group
NZC = NGRP * NCH        # 32 chunks total
ZS = 64.0               # fp8 weight scale
NT = R // 128           # 5

FP32 = mybir.dt.float32
FP16 = mybir.dt.float16
FP8 = mybir.dt.float8e4
AF = mybir.ActivationFunctionType
ALU = mybir.AluOpType
DR = mybir.MatmulPerfMode.DoubleRow
NPF8 = ml_dtypes.float8_e4m3


def bcast_ap(dram, parts, n):
    """DRAM [n] -> AP [[0,parts],[1,n]] (partition broadcast)."""
    a = dram[:]
    return bass.AP(tensor=a.tensor, offset=a.offset, ap=[[0, parts], [1, n]])


def build_core_kernel(use_cb=True):
    nc = bacc.Bacc(None, target_bir_lowering=False)

    d_z8 = nc.dram_tensor("z8", [NGRP, CZ, 2, RG], FP8, kind="ExternalInput")
    d_a = nc.dram_tensor("a_h", [R, CA], FP16, kind="ExternalInput")
    d_s = nc.dram_tensor("s_h", [R, CS], FP16, kind="ExternalInput")
    d_wq = nc.dram_tensor("wq_pad", [CA, 128 * KH], FP8, kind="ExternalInput")
    d_wk = nc.dram_tensor("wk_pad", [CA, 128 * KH], FP8, kind="ExternalInput")
    d_wv = nc.dram_tensor("wv", [CA, CA], FP16, kind="ExternalInput")
    d_wg = nc.dram_tensor("wg", [CA, CA], FP16, kind="ExternalInput")
    d_wo = nc.dram_tensor("wo", [CA, CA], FP16, kind="ExternalInput")
    d_agw = nc.dram_tensor("adaln_g_w", [CS, CA], FP16, kind="ExternalInput")
    d_asw = nc.dram_tensor("adaln_s_w", [CS, CA], FP16, kind="ExternalInput")
    d_wl = nc.dram_tensor("w_last", [CS, CA], FP16, kind="ExternalInput")
    d_agb = nc.dram_tensor("adaln_g_b", [128, KA], FP32, kind="ExternalInput")
    d_bo = nc.dram_tensor("bo_b", [128, KA], FP32, kind="ExternalInput")
    d_bl = nc.dram_tensor("b_last_b", [128, KA], FP32, kind="ExternalInput")
    d_bgf = nc.dram_tensor("bg_full", [CA], FP16, kind="ExternalInput")
    d_wzdr = nc.dram_tensor("wzdr", [CZ, 2, 18], FP8, kind="ExternalInput")
    d_csI = nc.dram_tensor("csI", [128, H, 128], FP16, kind="ExternalInput")
    d_mask = nc.dram_tensor("mask_nq", [128, 2, NK], FP16, kind="ExternalInput")
    d_cbr = nc.dram_tensor("cb_row", [1, H, NK], FP16, kind="ExternalInput")
    d_out = nc.dram_tensor("outT", [CA, ROWS], FP16, kind="ExternalOutput")

    with tile.TileContext(nc) as tc, ExitStack() as ctx:
        const = ctx.enter_context(tc.tile_pool(name="const", bufs=1))
        pers = ctx.enter_context(tc.tile_pool(name="pers", bufs=1))
        ln_p = ctx.enter_context(tc.tile_pool(name="ln", bufs=2))
        at_p = ctx.enter_context(tc.tile_pool(name="attn", bufs=2))
        zs_p = ctx.enter_context(tc.tile_pool(name="zs", bufs=4))
        psA = ctx.enter_context(tc.tile_pool(name="psA", bufs=4, space="PSUM"))
        psB = ctx.enter_context(tc.tile_pool(name="psB", bufs=4, space="PSUM"))

        def pA(shape):
            return psA.tile(shape, FP32, tag="A", name="pA")

        def pB(shape, dt=FP32):
            return psB.tile(shape, dt, tag="B", name="pB")

        # ---------- constants ----------
        I16 = const.tile([128, 128], FP16, tag="I16")
        make_identity(nc, I16)
        wzdr = const.tile([CZ, 2, 18], FP8, tag="wzdr")
        nc.sync.dma_start(out=wzdr, in_=d_wzdr[:])
        csI = const.tile([128, H, 128], FP16, tag="csI")
        maskT = const.tile([128, 2, NK], FP16, tag="maskT")
        bob = const.tile([128, KA], FP32, tag="bob")
        blb = const.tile([128, KA], FP32, tag="blb")
        bg_bc = const.tile([128, CA], FP16, tag="bg_bc")
        cbrow = const.tile([1, H, NK], FP16, tag="cbrow")
        agb = const.tile([128, KA], FP32, tag="agb")
        nc.sync.dma_start(out=agb, in_=d_agb[:])
        epsb = const.tile([128, 1], FP32, tag="epsb")
        nc.vector.memset(epsb, EPS)
        ones1 = const.tile([1, 128], FP16, tag="ones1")
        nc.vector.memset(ones1, 1.0)
        onesc = const.tile([128, 1], FP16, tag="onesc")
        nc.vector.memset(onesc, 1.0)

        def load_w8(pool, dram, kt, name):
            t = pool.tile([128, kt, dram.shape[1]], FP8, tag=name)
            nc.sync.dma_start(out=t, in_=dram[:].rearrange("(k p) c -> p k c", p=128))
            return t

        def load_w(pool, dram, kt, name):
            t = pool.tile([128, kt, dram.shape[1]], FP16, tag=name)
            nc.sync.dma_start(out=t, in_=dram[:].rearrange("(k p) c -> p k c", p=128))
            return t

        # ---------- persistent activations ----------
        sT = pers.tile([128, KS, R], FP16, tag="sT")
        qT_hd = pers.tile([128, KH, ROWS], FP16, tag="qT_hd")
        kT_hd = pers.tile([128, KH, R], FP16, tag="kT_hd")
        vw = pers.tile([128, BPC, CA], FP16, tag="vw")
        g_rm = pers.tile([128, NGRP, CA], FP16, tag="g_rm")
        go_rm = pers.tile([128, NGRP, CA], FP16, tag="go_rm")
        goT = pers.tile([128, KA, ROWS], FP16, tag="goT")
        gs16 = pers.tile([128, KA, ROWS], FP16, tag="gs16")
        bias_hm = pers.tile([128, NGRP, 18, NK], FP16, tag="bias_hm")
        p16_all = pers.tile([128, H * NGRP // 2, 256], FP16, tag="p16_all")
        mid_cm = tc.tile_pool(name="mid", bufs=1)
        mid = mid_cm.__enter__()
        a2T = mid.tile([128, KA, R], FP16, tag="a2T")
        a2T8 = mid.tile([128, KA, R], FP8, tag="a2T8")

        # ---------- interleaved z pipeline ----------
        # One step: DMA one [128, 2, CH] fp8 chunk (SP HWDGE), 16 DoubleRow
        # matmuls -> [128, 16, 18] PSUM, one transposing eviction into the
        # h-major bias tile. Steps are sprinkled through phases 1-2 so the z
        # HBM traffic hides under projection compute.
        zstate = {"i": 0, "ld": 0}
        zfifo = []

        def z_load():
            ld = zstate["ld"]
            if ld >= NZC:
                return
            zstate["ld"] = ld + 1
            g, cc = ld // NCH, ld % NCH
            zc = zs_p.tile([128, 2, CH], FP8, tag="zc")
            nc.sync.dma_start(out=zc, in_=d_z8[g, :, :, cc * CH : (cc + 1) * CH])
            zfifo.append(zc)

        def z_step(n=1):
            for _ in range(n):
                i = zstate["i"]
                if i >= NZC:
                    return
                zstate["i"] = i + 1
                g, cc = i // NCH, i % NCH
                if not zfifo:
                    z_load()
                zc = zfifo.pop(0)
                z_load()
                nsub = CH // 128
                half = nsub // 2
                for hi in range(2):
                    pz = pB([128, half, 18])
                    for k in range(half):
                        kk = hi * half + k
                        nc.tensor.matmul(
                            pz[:, k, :], zc[:, :, kk * 128 : (kk + 1) * 128], wzdr,
                            start=True, stop=True, perf_mode=DR,
                        )
                    k0 = cc * nsub + hi * half
                    if (2 * i + hi) % 2 < 1:
                        nc.vector.tensor_copy(
                            out=bias_hm[:, g, :, k0 : k0 + half],
                            in_=pz.rearrange("p k c -> p c k"),
                        )
                    else:
                        nc.scalar.activation(
                            out=bias_hm[:, g, :, k0 : k0 + half],
                            in_=pz.rearrange("p k c -> p c k"), func=AF.Copy,
                        )

        # ================= Phase 1: LN + adaln -> a2T =================
        with tc.tile_pool(name="ph1", bufs=1) as ph1:
            w_ag = load_w(ph1, d_agw, KS, "w_ag")
            w_as = load_w(ph1, d_asw, KS, "w_as")
            s_lnT = ph1.tile([128, KS, R], FP16, tag="s_lnT")
            a_lnT = ph1.tile([128, KA, R], FP16, tag="a_lnT")

            # Pass 1: load all row tiles, accumulate bn stats; ONE Sqrt for
            # every tile (keeps the Act table in one function era), then
            # normalize + transpose per tile in pass 2.
            srcs = [(d_s, CS, KS, s_lnT), (d_a, CA, KA, a_lnT)]
            xs = {}
            mv_all = ph1.tile([128, 2, NT, 2], FP32, tag="mv_all")
            for t in range(NT):
                for wi, (dram, width, kt, dst) in enumerate(srcs):
                    x = ph1.tile([128, width], FP16, tag=f"ln_x{width}", bufs=NT)
                    nc.sync.dma_start(out=x, in_=dram[t * 128 : (t + 1) * 128, :])
                    nsub = width // 384
                    st = ph1.tile([128, nsub, 6], FP32, tag=f"ln_st{width}", bufs=2)
                    for j in range(nsub):
                        nc.vector.bn_stats(
                            out=st[:, j, :], in_=x[:, j * 384 : (j + 1) * 384]
                        )
                    nc.vector.bn_aggr(out=mv_all[:, wi, t, :], in_=st)
                    xs[(wi, t)] = x
                z_step()
            # raw s transposed (all R rows, tile-aligned) for the final
            # gate, sourced from the pass-1 tiles before in-place normalize
            for t in range(NT):
                pt = pB([128, KS, 128], FP16)
                for k in range(KS):
                    nc.tensor.transpose(
                        pt[:, k, :], xs[(0, t)][:, k * 128 : (k + 1) * 128], I16
                    )
                nc.vector.tensor_copy(out=sT[:, :, t * 128 : (t + 1) * 128], in_=pt)

            rstd_ln = ph1.tile([128, 2, NT], FP32, tag="rstd_ln")
            nc.scalar.activation(
                out=rstd_ln, in_=mv_all[:, :, :, 1], func=AF.Sqrt, bias=epsb
            )
            nc.vector.reciprocal(out=rstd_ln, in_=rstd_ln)
            for t in range(NT):
                for wi, (dram, width, kt, dst) in enumerate(srcs):
                    x16 = xs[(wi, t)]
                    nc.vector.tensor_scalar(
                        out=x16, in0=x16, scalar1=mv_all[:, wi, t, 0:1],
                        scalar2=rstd_ln[:, wi, t : t + 1],
                        op0=ALU.subtract, op1=ALU.mult,
                    )
                    pt = pB([128, kt, 128], FP16)
                    for k in range(kt):
                        nc.tensor.transpose(
                            pt[:, k, :], x16[:, k * 128 : (k + 1) * 128], I16
                        )
                    nc.vector.tensor_copy(
                        out=dst[:, :, t * 128 : (t + 1) * 128], in_=pt
                    )
                z_step()


            RC = 320
            for co in range(KA):
                for rc in range(R // RC):
                    rs = slice(rc * RC, (rc + 1) * RC)
                    pg = pA([128, RC])
                    pv = pA([128, RC])
                    for k in range(KS):
                        nc.tensor.matmul(
                            pg, (w_ag[:, k, co * 128 : (co + 1) * 128]),
                            (s_lnT[:, k, rs]), start=(k == 0), stop=(k == KS - 1),
                        )
                    for k in range(KS):
                        nc.tensor.matmul(
                            pv, (w_as[:, k, co * 128 : (co + 1) * 128]),
                            (s_lnT[:, k, rs]), start=(k == 0), stop=(k == KS - 1),
                        )
                    gate = ph1.tile([128, RC], FP16, tag="gate", bufs=2)
                    nc.scalar.activation(
                        out=gate, in_=pg, func=AF.Sigmoid, bias=agb[:, co : co + 1]
                    )
                    nc.vector.tensor_mul(gate, gate, a_lnT[:, co, rs])
                    nc.vector.tensor_add(a2T[:, co, rs], gate, pv)
                    nc.gpsimd.tensor_copy(out=a2T8[:, co, rs], in_=a2T[:, co, rs])
                    z_step()

        nc.sync.dma_start(out=blb, in_=d_bl[:])
        # ============ Phase 1b: final-gate projection (sigmoid era) ==========
        with tc.tile_pool(name="ph1b", bufs=1) as ph1b:
            w_l = load_w(ph1b, d_wl, KS, "w_l")
            for co in range(KA):
                pl = pA([128, ROWS])
                for k in range(KS):
                    nc.tensor.matmul(
                        pl, (w_l[:, k, co * 128 : (co + 1) * 128]),
                        (sT[:, k, HALO : HALO + ROWS]),
                        start=(k == 0), stop=(k == KS - 1),
                    )
                nc.scalar.activation(
                    out=gs16[:, co, :], in_=pl, func=AF.Sigmoid,
                    bias=blb[:, co : co + 1],
                )
                z_step()

        # ================= Phase 2a: q/k projections (head-aligned) ==========
        with tc.tile_pool(name="ph2a", bufs=1) as ph2a:
            w_q = load_w8(ph2a, d_wq, KA, "w_q")
            w_k = load_w8(ph2a, d_wk, KA, "w_k")
            wq4 = w_q.rearrange("p (kp two) c -> p kp two c", two=2)
            wk4 = w_k.rearrange("p (kp two) c -> p kp two c", two=2)
            a84 = a2T8.rearrange("p (kp two) r -> p kp two r", two=2)
            for co in range(KH):
                pq = [pA([128, 256]) for _ in range(2)]
                pk = [pA([128, 320]) for _ in range(2)]
                for kp in range(KA // 2):
                    for rc in range(2):
                        rs = slice(HALO + rc * 256, HALO + (rc + 1) * 256)
                        nc.tensor.matmul(
                            pq[rc], (wq4[:, kp, :, co * 128 : (co + 1) * 128]),
                            (a84[:, kp, :, rs]), start=(kp == 0),
                            stop=(kp == KA // 2 - 1), perf_mode=DR,
                        )
                for rc in range(2):
                    if co % 2 == 0:
                        nc.scalar.activation(
                            out=qT_hd[:, co, rc * 256 : (rc + 1) * 256], in_=pq[rc],
                            func=AF.Copy, scale=1.0 / ZS,
                        )
                    else:
                        nc.vector.tensor_scalar(
                            out=qT_hd[:, co, rc * 256 : (rc + 1) * 256], in0=pq[rc],
                            scalar1=1.0 / ZS, scalar2=None, op0=ALU.mult,
                        )
                for kp in range(KA // 2):
                    for rc in range(2):
                        rs = slice(rc * 320, (rc + 1) * 320)
                        nc.tensor.matmul(
                            pk[rc], (wk4[:, kp, :, co * 128 : (co + 1) * 128]),
                            (a84[:, kp, :, rs]), start=(kp == 0),
                            stop=(kp == KA // 2 - 1), perf_mode=DR,
                        )
                if co % 2 == 0:
                    for rc in range(2):
                        nc.scalar.activation(
                            out=kT_hd[:, co, rc * 320 : (rc + 1) * 320], in_=pk[rc],
                            func=AF.Copy, scale=1.0 / ZS,
                        )
                else:
                    for rc in range(2):
                        nc.vector.tensor_scalar(
                            out=kT_hd[:, co, rc * 320 : (rc + 1) * 320], in0=pk[rc],
                            scalar1=1.0 / ZS, scalar2=None, op0=ALU.mult,
                        )
                z_step()

        nc.sync.dma_start(out=bg_bc, in_=bcast_ap(d_bgf, 128, CA))
        # ================= Phase 2b: v (row-major) + g =================
        with tc.tile_pool(name="ph2b", bufs=1) as ph2b:
            w_v = load_w(ph2b, d_wv, KA, "w_v")
            w_g = load_w(ph2b, d_wg, KA, "w_g")
            v_rm = ph2b.tile([128, NT, CA], FP16, tag="v_rm")
            for rt in range(NT):
                for c2 in range(2):
                    pv = pA([128, 384])
                    for k in range(KA):
                        nc.tensor.matmul(
                            pv, (a2T[:, k, rt * 128 : (rt + 1) * 128]),
                            (w_v[:, k, c2 * 384 : (c2 + 1) * 384]),
                            start=(k == 0), stop=(k == KA - 1),
                        )
                    nc.vector.tensor_copy(
                        out=v_rm[:, rt, c2 * 384 : (c2 + 1) * 384], in_=pv
                    )
                z_step(2)
            for rt in range(ROWS // 128):
                for c2 in range(2):
                    pg = pA([128, 384])
                    for k in range(KA):
                        nc.tensor.matmul(
                            pg, (a2T[:, k, HALO + rt * 128 : HALO + (rt + 1) * 128]),
                            (w_g[:, k, c2 * 384 : (c2 + 1) * 384]),
                            start=(k == 0), stop=(k == KA - 1),
                        )
                    nc.vector.tensor_add(pg, pg, bg_bc[:, c2 * 384 : (c2 + 1) * 384])
                    nc.scalar.activation(
                        out=g_rm[:, rt, c2 * 384 : (c2 + 1) * 384], in_=pg,
                        func=AF.Sigmoid,
                    )
                z_step()
            # per-block key/value windows of v: 4 batched strided copies per
            # piece (blocks n=4t+j share partition offset p0=16+32j, t=0..3).
            # Pool SWDGE so these don't block the z stream on SP's queue.
            vw4 = vw.rearrange("p (t j) c -> p t j c", j=4)
            for j in range(4):
                p0 = 16 + 32 * j
                n0 = 128 - p0
                nc.gpsimd.dma_start(out=vw4[0:n0, :, j, :], in_=v_rm[p0:128, 0:4, :])
                nc.gpsimd.dma_start(out=vw4[n0:128, :, j, :], in_=v_rm[0:p0, 1:5, :])
            z_step(NZC)  # drain whatever z remains

        mid_cm.__exit__(None, None, None)

        nc.sync.dma_start(out=csI, in_=d_csI[:])
        nc.sync.dma_start(out=maskT, in_=d_mask[:])
        nc.sync.dma_start(out=cbrow, in_=d_cbr[:])
        nc.sync.dma_start(out=bob, in_=d_bo[:])
        w_o = load_w(pers, d_wo, KA, "w_o")
        fin_sb = pers.tile([128, KA, ROWS], FP16, tag="fin_sb")

        # ================= Phase 3: group stats =================
        # mu/ex2 from the x64-scaled sum columns; rstd via Rsqrt (one act
        # table load for all four groups); rep2 is rstd/64 duplicated per
        # head-pair lane for the packed th multiply.
        murs, reps = [], []
        mus = []
        var_all = at_p.tile([128, NGRP, NK], FP32, tag="var_all", bufs=1)
        for g in range(NGRP):
            mu = at_p.tile([128, NK], FP32, tag="mu", bufs=4)
            nc.scalar.mul(out=mu, in_=bias_hm[:, g, 16, :], mul=1.0 / (ZS * CZ))
            ex2 = at_p.tile([128, NK], FP32, tag="ex2")
            nc.scalar.mul(out=ex2, in_=bias_hm[:, g, 17, :], mul=1.0 / (ZS * CZ))
            nc.vector.tensor_mul(var_all[:, g, :], mu, mu)
            nc.vector.tensor_sub(var_all[:, g, :], ex2, var_all[:, g, :])
            mus.append(mu)
        rstd_all = at_p.tile([128, NGRP, NK], FP32, tag="rstd_all", bufs=1)
        nc.scalar.activation(out=rstd_all, in_=var_all, func=AF.Sqrt, bias=epsb)
        nc.vector.reciprocal(out=rstd_all, in_=rstd_all)
        for g in range(NGRP):
            mur = at_p.tile([128, NK], FP16, tag="mur", bufs=4)
            nc.vector.tensor_mul(mur, mus[g], rstd_all[:, g, :])
            rep4 = at_p.tile([128, 4, NK], FP16, tag="rep4", bufs=4)
            for u in range(4):
                nc.scalar.mul(out=rep4[:, u, :], in_=rstd_all[:, g, :], mul=1.0 / ZS)
            murs.append(mur)
            reps.append(rep4)

        # ================= Phase 4: attention L1 — scores -> p16 ============
        # Quad-batched: one [128, 4, NK] score tile, one bias multiply, one
        # add and one exp per 4 heads (= 2 pairs = half a group-row).
        pairs = [(g, hp) for g in range(NGRP) for hp in range(KH)]
        for qq in range(len(pairs) // 2):
            g, hp0 = pairs[2 * qq]
            edge = g in (0, NGRP - 1)
            S4 = pB([128, 4, NK])
            for jj in range(4):
                h = 4 * (qq % (KH // 2)) + jj if False else 2 * hp0 + jj
                hp = h // 2
                o64 = (h % 2) * 64
                for nn in range(4):
                    n = g * 4 + nn
                    wlo = 16 + 32 * n
                    nc.tensor.matmul(
                        S4[nn * NQ : (nn + 1) * NQ, jj, :],
                        qT_hd[o64 : o64 + DH, hp, n * NQ : (n + 1) * NQ],
                        kT_hd[o64 : o64 + DH, hp, wlo : wlo + NK],
                        start=True, stop=False,
                        tile_position=(o64, nn * NQ),
                    )
                nc.tensor.matmul(
                    S4[:, jj, :], csI[:, h, :], murs[g], start=False,
                    stop=not (use_cb or edge),
                )
                if use_cb:
                    nc.tensor.matmul(
                        S4[:, jj, :], ones1, cbrow[:, h, :], start=False,
                        stop=not edge,
                    )
                if edge:
                    nc.tensor.matmul(
                        S4[:, jj, :], I16, maskT[:, 0 if g == 0 else 1, :],
                        start=False, stop=True,
                    )
            th4 = at_p.tile([128, 4, NK], FP16, tag="th4", bufs=2)
            nc.vector.tensor_mul(
                th4, bias_hm[:, g, 2 * hp0 : 2 * hp0 + 4, :], reps[g]
            )
            nc.vector.tensor_add(S4, S4, th4)
            nc.scalar.activation(
                out=p16_all[:, 2 * qq : 2 * qq + 2, :], in_=S4, func=AF.Exp
            )

        # ============ Phase 5: attention L2 — transpose, sums, AV ============
        # Processed as pair-pairs (4 heads): one PSUM transpose bank, one Act
        # eviction and one reciprocal per 2 pairs. l2_front of pair-pair p+2
        # issues before l2_back(p) so PE never stalls behind the Act copy.
        l2fin = []
        pT4s = {}
        NPP = len(pairs) // 2

        def l2_front(pp):
            ptp4 = pB([128, 4, 128], FP16)
            for u in range(2):
                pi = 2 * pp + u
                p16v = p16_all[:, pi, :].rearrange("p (j k) -> p j k", j=2)
                for j in range(2):
                    nc.tensor.transpose(ptp4[:, 2 * u + j, :], p16v[:, j, :], I16)
            pT4 = at_p.tile([128, 4, 128], FP16, tag="pT4", bufs=3)
            nc.scalar.activation(out=pT4, in_=ptp4, func=AF.Copy)
            pT4s[pp] = pT4

        def l2_back(pp):
            pT4 = pT4s.pop(pp)
            sums4 = pA([128, 4])
            ov4 = pA([128, 4, DH])
            for u in range(2):
                pi = 2 * pp + u
                g, hp = pairs[pi]
                for j in range(2):
                    h = 2 * hp + j
                    jj = 2 * u + j
                    nc.tensor.matmul(
                        sums4[:, jj : jj + 1], pT4[:, jj, :], onesc,
                        start=True, stop=True,
                    )
                    for nn in range(4):
                        n = g * 4 + nn
                        nc.tensor.matmul(
                            ov4[nn * NQ : (nn + 1) * NQ, jj, :],
                            pT4[:, jj, nn * NQ : (nn + 1) * NQ],
                            vw[:, n, h * DH : (h + 1) * DH],
                            start=True, stop=True,
                            tile_position=(0, nn * NQ),
                        )
            rec4 = at_p.tile([128, 4], FP32, tag="rec4", bufs=3)
            nc.vector.reciprocal(out=rec4, in_=sums4)
            l2fin.append((pp, ov4, rec4))
            while l2fin and (l2fin[0][0] < pp or pp == NPP - 1):
                fpp, fov4, frec4 = l2fin.pop(0)
                for u in range(2):
                    fpi = 2 * fpp + u
                    fg, fhp = pairs[fpi]
                    for j in range(2):
                        h = 2 * fhp + j
                        jj = 2 * u + j
                        nc.vector.scalar_tensor_tensor(
                            out=go_rm[:, fg, h * DH : (h + 1) * DH],
                            in0=fov4[:, jj, :],
                            scalar=frec4[:, jj : jj + 1],
                            in1=g_rm[:, fg, h * DH : (h + 1) * DH],
                            op0=ALU.mult, op1=ALU.mult,
                        )

        for pp in range(NPP):
            l2_front(pp)
            if pp >= 2:
                l2_back(pp - 2)
        l2_back(NPP - 2)
        l2_back(NPP - 1)

        # ================= Phase 6: (g*o) transpose =================
        for g in range(NGRP):
            pt = pB([128, KA, 128], FP16)
            for k in range(KA):
                nc.tensor.transpose(pt[:, k, :], go_rm[:, g, k * 128 : (k + 1) * 128], I16)
            nc.scalar.activation(
                out=goT[:, :, g * 128 : (g + 1) * 128], in_=pt, func=AF.Copy
            )

        # ================= Phase 7: output projection =================
        for co in range(KA):
            po = pA([128, ROWS])
            for k in range(KA):
                nc.tensor.matmul(
                    po, (w_o[:, k, co * 128 : (co + 1) * 128]),
                    (goT[:, k, :]), start=(k == 0), stop=(k == KA - 1),
                )
            nc.vector.scalar_tensor_tensor(
                out=fin_sb[:, co, :], in0=po, scalar=bob[:, co : co + 1],
                in1=gs16[:, co, :], op0=ALU.add, op1=ALU.mult,
            )

        # ================= Phase 7 tail: stores =================
        for co in range(KA):
            nc.sync.dma_start(
                out=d_out[co * 128 : (co + 1) * 128, :], in_=fin_sb[:, co, :]
            )

    nc.compile()
    return nc


def host_prep(inputs):
    a = np.ascontiguousarray(np.asarray(inputs["a"], np.float32)[0])
    s = np.ascontiguousarray(np.asarray(inputs["s"], np.float32)[0])
    z = np.asarray(inputs["z"], np.float32)[0]
    gz = np.asarray(inputs["gz"], np.float32)
    bz = np.asarray(inputs["bz"], np.float32)
    wz = np.asarray(inputs["wz"], np.float32)
    wz2 = gz[:, None] * wz
    cs = wz2.sum(0)
    cb = (bz @ wz).astype(np.float32)

    # DoubleRow pair-bias weights: [CZ, 2, 18] fp8, x64 for fp8 range.
    # Slot 0 pairs with z (head cols + sum col), slot 1 with z^2 (sum col).
    wzdr = np.zeros((CZ, 2, 18), np.float32)
    wzdr[:, 0, :H] = wz2 * ZS
    wzdr[:, 0, 16] = ZS
    wzdr[:, 1, 17] = ZS
    wzdr = wzdr.astype(NPF8)

    csI = np.zeros((128, H, 128), np.float16)
    for h in range(H):
        for p in range(128):
            csI[p, h, p] = np.float16(-cs[h])

    cb_row = np.zeros((1, H, NK), np.float16)
    cb_row[0, :, :] = cb[:, None]

    # head-aligned padded projection weights: head h -> cols (h//2)*128 +
    # (h%2)*64 + [0,48); 1/sqrt(dh) folded into wq
    def pad_heads(w, scale=1.0):
        wp = np.zeros((CA, 128 * KH), np.float32)
        for h in range(H):
            dst = (h // 2) * 128 + (h % 2) * 64
            wp[:, dst : dst + DH] = w[:, h * DH : (h + 1) * DH] * scale
        return wp.astype(np.float16)

    def btile(v):
        return np.ascontiguousarray(v.reshape(KA, 128).T.astype(np.float32))

    common = {
        "wq_pad": pad_heads(np.asarray(inputs["wq"], np.float32), ISCALE * ZS).astype(np.float32).astype(NPF8),
        "wk_pad": pad_heads(np.asarray(inputs["wk"], np.float32), ZS).astype(np.float32).astype(NPF8),
        "wv": np.asarray(inputs["wv"], np.float16),
        "wg": np.asarray(inputs["wg"], np.float16),
        "wo": np.asarray(inputs["wo"], np.float16),
        "adaln_g_w": np.asarray(inputs["adaln_g_w"], np.float16),
        "adaln_s_w": np.asarray(inputs["adaln_s_w"], np.float16),
        "w_last": np.asarray(inputs["w_last"], np.float16),
        "adaln_g_b": btile(np.asarray(inputs["adaln_g_b"], np.float32)),
        "bo_b": btile(np.asarray(inputs["bo"], np.float32)),
        "b_last_b": btile(np.asarray(inputs["b_last"], np.float32)),
        "bg_full": np.asarray(inputs["bg"], np.float16),
        "wzdr": wzdr, "csI": csI, "cb_row": cb_row,
    }

    in_maps = []
    for c in range(NCORE):
        lo = c * ROWS - HALO
        hi = c * ROWS + ROWS + HALO
        a_h = np.zeros((R, CA), np.float16)
        s_h = np.zeros((R, CS), np.float16)
        g0, g1 = max(lo, 0), min(hi, N)
        a_h[g0 - lo : g1 - lo] = a[g0:g1]
        s_h[g0 - lo : g1 - lo] = s[g0:g1]
        z_c = z[c * BPC : (c + 1) * BPC]
        zg = z_c.reshape(NGRP, 4, NQ, NK, CZ)
        # [NGRP, CZ, (k, n, q)] position-major, fp8; z^2 from the fp8-rounded z
        zT = zg.transpose(0, 4, 3, 1, 2).reshape(NGRP, CZ, RG).astype(NPF8)
        z8 = np.empty((NGRP, CZ, 2, RG), NPF8)
        z8[:, :, 0, :] = zT
        z8[:, :, 1, :] = (zT.astype(np.float32) ** 2).astype(NPF8)
        z8 = np.ascontiguousarray(z8)
        nglob = c * BPC + np.arange(BPC)
        idx = nglob[:, None] * NQ + np.arange(NK)[None, :] - OFF
        mask = np.where((idx >= 0) & (idx < N), 0.0, -30000.0).astype(np.float32)
        # edge groups only: [128(4n,32q), {first,last}, NK]
        mask_g = (
            np.repeat(mask.reshape(NGRP, 4, 1, NK), NQ, axis=2)
            .reshape(NGRP, 128, NK)
            .transpose(1, 0, 2)
            .astype(np.float16)
        )
        mask_nq = np.ascontiguousarray(mask_g[:, [0, NGRP - 1], :])
        m = dict(common)
        m.update({"z8": z8, "a_h": a_h, "s_h": s_h, "mask_nq": mask_nq})
        in_maps.append(m)
    return in_maps


_NC_CACHE = {}


def kernel(**inputs):
    use_cb = bool(
        np.any(np.asarray(inputs["bz"], np.float32) @ np.asarray(inputs["wz"], np.float32))
    )
    key = ("nc", use_cb)
    if key not in _NC_CACHE:
        _NC_CACHE[key] = build_core_kernel(use_cb=use_cb)
    nc = _NC_CACHE[key]
    in_maps = host_prep(inputs)
    res = bass_utils.run_bass_kernel_spmd(
        nc, in_maps, core_ids=list(range(NCORE)),
        trace=bool(int(os.environ.get("KTRACE", "0"))),
    )
    kernel.last_results = res
    outs = [np.asarray(res.results[c]["outT"]).T for c in range(NCORE)]
    return np.ascontiguousarray(np.concatenate(outs, 0)[None]).astype(np.float32)
